# revision 1
# baseline (speedup 1.0000x reference)
"""Trainium2 Bass kernel for nn_DecoderMultiHeadAttention_38293928411157.

Full (unsharded) inputs in, full output out. Cores: (batch x head-group);
core c handles batch c//2 and heads [(c%2)*8, (c%2)*8+8). Host sums the two
row-shard partials per batch.

Design (ScalarE-bound; everything else hides behind the exp stream):
  - Host pre-transposes q/k/v to [D, S] and uploads bf16: no PE transposes
    or PSUM evictions for the inputs at all.
  - QK^T streams into [128,1536] PSUM tiles (3 banks, double buffered);
    one ScalarE exp per tile amortizes the access-latency overhead.
    ScalarE is the bottleneck engine (~251us busy of ~301us total).
  - PV is flipped to out[s,e]: lhsT = exp-scores block [t,s], rhs = V
    [t,33] bf16 (32 dims + a ones column producing the softmax row sums);
    each PV matmul costs 33 output columns instead of 512.
  - 8 PV accumulators pack into ONE PSUM bank at 33-column offsets. They
    never use start=True (a start clears the has_written state of the
    whole bank, wiping sibling accumulators) — a DVE memset zeroes the
    region between rounds and the matmuls accumulate onto it.
  - All projections (K/V/Q chunks) and the output projection run as
    'extras': 256-wide PE groups through 2 rotating [128,256] util-PSUM
    slots, emitted from a deadline heap. A group MUST be emitted before
    the global exp-tile index that first consumes its output, otherwise
    the consumer precedes it in program order and the tile framework sees
    no RAW dependency (the HW then reads garbage).
  - PV/normalize/WO trail the QK/exp stream via a global queue (deep lag
    while the input DMAs stream in, shallower afterwards); the last
    s-block's output projection runs wide in the freed QK PSUM banks.
PSUM: 6 banks QK scores, 1 bank PV accumulators, 1 bank util slots.
GPSIMD cannot read PSUM on real HW — all PSUM evictions are DVE-only.
"""

import os
import sys

sys.path.insert(0, "/opt/trn_rl_repo")

import numpy as np

import concourse.bacc as bacc
import concourse.tile as tile
from concourse import mybir
from concourse.bass_utils import run_bass_kernel_spmd

F32 = mybir.dt.float32
F32R = mybir.dt.float32r
BF16 = mybir.dt.bfloat16
EXP = mybir.ActivationFunctionType.Exp

D = 1024            # d_model
S = 2048            # sequence length
B = 4               # batch
NH = 16             # total heads
DH = 32             # head dim
HPC = 8             # heads per core
E = HPC * DH        # 256: concat feature dim per core
VSTRIDE = 40        # per-head column stride in V_aug
SCALE = 1.0 / 32.0  # d_model ** -0.5

N_CORES = 8
DC = D // 128       # 8 d-model chunks
SC = S // 512       # 4 s chunks
TC = S // 128       # 16 t blocks
N_ROUNDS = 16       # (s_q, pair)
UPR = 32            # units per round: (t_blk, hl)
TPU = 3             # units per exp tile
TILES_PER_ROUND = 11  # 10 full (3 units) + 1 runt (2 units)

_CACHED_NC = None
AT_BUFS = int(os.environ.get("KRN_AT_BUFS", "22"))
EXTRA_BUDGET = int(os.environ.get("KRN_EXTRA_BUDGET", "1"))
PV_LAG = int(os.environ.get("KRN_PV_LAG", "18"))
WARMUP_TRANSPOSES = int(os.environ.get("KRN_WARMUP", "24"))
WARM_LAG = int(os.environ.get("KRN_WARM_LAG", "21"))


def _build_nc():
    nc = bacc.Bacc("TRN2", target_bir_lowering=False, debug=False,
                   num_devices=N_CORES)

    qt_d = nc.declare_dram_parameter("qt", [D, S], BF16, isOutput=False)
    kt_d = nc.declare_dram_parameter("kt", [D, S], BF16, isOutput=False)
    vt_d = nc.declare_dram_parameter("vt", [D, S], BF16, isOutput=False)
    wq_d = nc.declare_dram_parameter("wq", [D, E], BF16, isOutput=False)
    wk_d = nc.declare_dram_parameter("wk", [D, E], BF16, isOutput=False)
    wv_d = nc.declare_dram_parameter("wv", [D, E], BF16, isOutput=False)
    wo_d = nc.declare_dram_parameter("wo", [E, D], BF16, isOutput=False)
    id_d = nc.declare_dram_parameter("ident", [128, 128], F32R, isOutput=False)
    ones_d = nc.declare_dram_parameter("ones8", [128, HPC], BF16, isOutput=False)
    out_d = nc.declare_dram_parameter("out", [S, D], BF16, isOutput=True)

    with tile.TileContext(nc) as tc:
        _emit(nc, tc, qt_d, kt_d, vt_d, wq_d, wk_d, wv_d, wo_d, id_d, ones_d, out_d)
    nc.compile()
    return nc


def _emit(nc, tc, qt_d, kt_d, vt_d, wq_d, wk_d, wv_d, wo_d, id_d, ones_d, out_d):
    import contextlib

    ctx = contextlib.ExitStack()
    with ctx:
        const_p = ctx.enter_context(tc.tile_pool(name="const", bufs=1))
        w_p = ctx.enter_context(tc.tile_pool(name="w", bufs=1))
        nat_p = ctx.enter_context(tc.tile_pool(name="nat", bufs=9))
        quad_p = ctx.enter_context(tc.tile_pool(name="quad", bufs=1))
        vaug_p = ctx.enter_context(tc.tile_pool(name="vaug", bufs=TC))
        at_p = ctx.enter_context(tc.tile_pool(name="at", bufs=AT_BUFS))
        xsb_p = ctx.enter_context(tc.tile_pool(name="xsb", bufs=8))
        xt_p = ctx.enter_context(tc.tile_pool(name="xt", bufs=4))
        outsb_p = ctx.enter_context(tc.tile_pool(name="outsb", bufs=4))
        small_p = ctx.enter_context(tc.tile_pool(name="small", bufs=4))
        ps_qk = ctx.enter_context(
            tc.tile_pool(name="ps_qk", bufs=2, space="PSUM"))
        ps_pv = ctx.enter_context(
            tc.tile_pool(name="ps_pv", bufs=1, space="PSUM"))
        ps_ut = ctx.enter_context(
            tc.tile_pool(name="ps_ut", bufs=1, space="PSUM"))

        ident = const_p.tile([128, 128], F32R, tag="ident")
        nc.sync.dma_start(ident[:], id_d[:])
        ones_t = const_p.tile([128, HPC], BF16, tag="ones")
        nc.sync.dma_start(ones_t[:], ones_d[:])

        def load_w(dram, name):
            wt = w_p.tile([128, DC * E], BF16, tag=f"w_{name}", name=name)
            src = dram[:].rearrange("(dc p) e -> p dc e", p=128)
            dst = wt[:].rearrange("p (dc e) -> p dc e", dc=DC)
            nc.sync.dma_start(dst, src)
            return wt

        def load_chunk(x_d, j, name):
            # xT chunk: [128, dc=8 x 512] <- x_d[dc*128: , j*512:+512]
            ch = nat_p.tile([128, DC * 512], BF16, tag="nat", name=name)
            src = x_d[:].rearrange("(dc p) s -> p dc s", p=128)[:, :, j * 512:(j + 1) * 512]
            dst = ch[:].rearrange("p (dc s) -> p dc s", dc=DC)
            nc.sync.dma_start(dst, src)
            return ch

        def load_chunk_split(x_d, j, name):
            # same chunk as two half-DMAs (dc 0-3 / 4-7) so projections can
            # start after the first half lands
            halves = []
            for hh in range(2):
                t = nat_p.tile([128, 4 * 512], BF16, tag="nath", bufs=4,
                               name=f"{name}_{hh}")
                src = x_d[:].rearrange("(dc p) s -> p dc s", p=128)[
                    :, hh * 4:(hh + 1) * 4, j * 512:(j + 1) * 512]
                dst = t[:].rearrange("p (dc s) -> p dc s", dc=4)
                nc.sync.dma_start(dst, src)
                halves.append(t)
            return tuple(halves)

        def chsl(ch, dc, a, b):
            if isinstance(ch, tuple):
                return ch[dc // 4][:, (dc % 4) * 512 + a:(dc % 4) * 512 + b]
            return ch[:, dc * 512 + a: dc * 512 + b]

        # util-slot tile (also the warmup target): 2 x [128,256] slots
        ut = ps_ut.tile([128, 512], F32, tag="ps_ut")
        ut_ctr = [0]

        def next_slot():
            s = ut_ctr[0] % 2
            ut_ctr[0] += 1
            return ut[:, s * 256:(s + 1) * 256]

        # DMA order matters: first K-chunk 0 + its weights so the PE can
        # start projecting ASAP; everything else after.
        wk_t = load_w(wk_d, "wk_t")
        kch0 = load_chunk(kt_d, 0, "kch0")
        wq_t = load_w(wq_d, "wq_t")
        qch0 = load_chunk(qt_d, 0, "qch0")
        wv_t = load_w(wv_d, "wv_t")
        kchs = {0: kch0}
        vchs = {}
        kchs[1] = load_chunk(kt_d, 1, "kch1")
        kchs[2] = load_chunk(kt_d, 2, "kch2")
        kchs[3] = load_chunk(kt_d, 3, "kch3")
        vchs[0] = load_chunk(vt_d, 0, "vch0")
        vchs[1] = load_chunk(vt_d, 1, "vch1")
        vchs[2] = load_chunk(vt_d, 2, "vch2")
        vchs[3] = load_chunk(vt_d, 3, "vch3")
        wo_t = [w_p.tile([128, D], BF16, tag=f"w_wo{i}", name=f"wo_t{i}")
                for i in range(2)]
        for ec in range(2):
            nc.sync.dma_start(wo_t[ec][:], wo_d[ec * 128:(ec + 1) * 128, :])

        kq = [quad_p.tile([128, S], BF16, tag=f"kq{i}", name=f"kq{i}")
              for i in range(2)]
        qq = [quad_p.tile([128, S], BF16, tag=f"qq{i}", name=f"qq{i}")
              for i in range(2)]
        vaug = [vaug_p.tile([128, HPC * VSTRIDE], BF16, tag="vaug",
                            name=f"vaug{t}") for t in range(TC)]
        for t in range(TC):
            vv = vaug[t][:].rearrange("p (h c) -> p h c", h=HPC)
            nc.sync.dma_start(vv[:, :, 32], ones_t[:])

        # PE warmup: keep the PE continuously busy through the DMA head so
        # the cost model's p-state ramp reaches full clock before K0.
        for _ in range(WARMUP_TRANSPOSES):
            nc.tensor.transpose(ut[:, 0:128].bitcast(F32R), ident[:], ident[:])

        # ---------------- wide-PSUM prelude projections ---------------------
        def proj_qk_wide(ch, w_t, quads, j):
            for quad in range(2):
                scr = ps_qk.tile([128, 1536], F32, tag="ps_qk", bufs=2)
                for dc in range(DC):
                    nc.tensor.matmul(
                        scr[:, 0:512],
                        w_t[:, dc * E + quad * 128: dc * E + quad * 128 + 128],
                        ch[:, dc * 512:(dc + 1) * 512],
                        start=(dc == 0), stop=(dc == DC - 1),
                    )
                nc.vector.tensor_copy(quads[quad][:, j * 512:(j + 1) * 512],
                                      scr[:, 0:512])

        # prelude projections: quad 0 of K chunk 0 / Q chunk 0 only — that's
        # all rounds 0-1 (pairs 0-1) touch; quad 1 rides along as extras.
        def proj_qk_wide_quad(ch, w_t, quads, j, quad):
            scr = ps_qk.tile([128, 1536], F32, tag="ps_qk", bufs=2)
            for dc in range(DC):
                nc.tensor.matmul(
                    scr[:, 0:512],
                    w_t[:, dc * E + quad * 128: dc * E + quad * 128 + 128],
                    chsl(ch, dc, 0, 512),
                    start=(dc == 0), stop=(dc == DC - 1),
                )
            nc.vector.tensor_copy(quads[quad][:, j * 512:(j + 1) * 512],
                                  scr[:, 0:512])

        proj_qk_wide_quad(kch0, wk_t, kq, 0, 0)
        proj_qk_wide_quad(qch0, wq_t, qq, 0, 0)
        proj_qk_wide_quad(kch0, wk_t, kq, 0, 1)

        # extras: deadline-ordered heap of (deadline_gt, seq, closure). A
        # closure MUST be emitted before the global-tile index where its
        # output is first consumed — otherwise the consumer precedes it in
        # program order, the framework sees no RAW dependency, and the HW
        # reads garbage.
        import heapq
        extras = []
        ex_seq = [0]
        ev_ctr = [0]

        def evict_engine():
            # GPSIMD cannot read PSUM on real HW — PSUM evicts are DVE-only
            ev_ctr[0] += 1
            return nc.vector

        def push_extra(dl, fn):
            heapq.heappush(extras, (dl, ex_seq[0], fn))
            ex_seq[0] += 1

        def emit_extras_at(gt):
            popped = 0
            while extras and extras[0][0] <= gt + 1:
                heapq.heappop(extras)[2]()
                popped += 1
            if popped == 0 and extras and EXTRA_BUDGET > 0:
                heapq.heappop(extras)[2]()

        def emit_extras(n):
            for _ in range(n):
                if not extras:
                    return
                heapq.heappop(extras)[2]()

        def proj_qk_extras(w_t, quads, ch, j, quads_sel=(0, 1),
                           dl_fn=None):
            # 256-wide groups: (quad, s-half of 256). dl_fn(half, quad) ->
            # global tile by which the group must be emitted.
            for half in range(2):
                for quad in quads_sel:
                    dl = dl_fn(half, quad)
                    def go(ch=ch, quad=quad, half=half, j=j):
                        slot = next_slot()
                        for dc in range(DC):
                            nc.tensor.matmul(
                                slot,
                                w_t[:, dc * E + quad * 128: dc * E + quad * 128 + 128],
                                chsl(ch, dc, half * 256, half * 256 + 256),
                                start=(dc == 0), stop=(dc == DC - 1),
                                skip_group_check=True,
                            )
                        evict_engine().tensor_copy(
                            quads[quad][:, j * 512 + half * 256: j * 512 + half * 256 + 256],
                            slot)
                    push_extra(dl, go)

        def k_chunk_extras(j, quads_sel=(0, 1)):
            # kq cols [j*512+half*256 ...) are first read by the QK tile
            # covering t_blk 4j+2*half — in round 0 (quad 0) / round 2
            # (quad 1)
            proj_qk_extras(
                wk_t, kq, kchs[j], j, quads_sel,
                dl_fn=lambda half, quad: (0 if quad == 0 else 22)
                + (2 * (4 * j + 2 * half)) // 3)

        def q_chunk_extras(j):
            ch = load_chunk(qt_d, j, f"qch{j}")
            # Q chunk j is first read at round 4j tile 0 (quad 0) / round
            # 4j+2 (quad 1)
            proj_qk_extras(
                wq_t, qq, ch, j,
                dl_fn=lambda half, quad: 44 * j + (0 if quad == 0 else 22))

        def v_chunk_extras(j):
            ch = vchs[j]
            for tb in range(4):
                # first PV pop reading vaug[4j+tb] trails the QK stream by
                # at least PV_LAG tiles
                dl = (2 * (4 * j + tb)) // 3 + PV_LAG - 1
                def go(ch=ch, tb=tb, j=j):
                    slot = next_slot()
                    for dc in range(DC):
                        nc.tensor.matmul(
                            slot,
                            chsl(ch, dc, tb * 128, tb * 128 + 128),
                            wv_t[:, dc * E:(dc + 1) * E],
                            start=(dc == 0), stop=(dc == DC - 1),
                            skip_group_check=True,
                        )
                    t = j * 4 + tb
                    dstv = vaug[t][:].rearrange("p (h c) -> p h c", h=HPC)
                    srcv = slot.rearrange("p (h c) -> p h c", c=32)
                    evict_engine().tensor_copy(dstv[:, :, 0:32], srcv[:, :, :])
                push_extra(dl, go)

        def wo_extras(s_q, x_sbs, xt2):
            # xt2 layout: [128 e, ec*512 + sb*128 + s] (both e-chunks)
            for sb in range(4):
                def go_t(sb=sb, s_q=s_q):
                    slot = next_slot()
                    for ec in range(2):
                        nc.tensor.transpose(
                            slot[:, ec * 128:(ec + 1) * 128].bitcast(F32R),
                            x_sbs[sb][:, ec * 128:(ec + 1) * 128],
                            ident[:],
                        )
                    dst = xt2[:].rearrange("p (ec s) -> p ec s", ec=2)[
                        :, :, sb * 128:(sb + 1) * 128]
                    src = slot.rearrange("p (ec c) -> p ec c", ec=2)
                    evict_engine().tensor_copy(dst, src)
                push_extra(10**9, go_t)
            for sb in range(4):
                ot = outsb_p.tile([128, D], BF16, tag="outsb",
                                  name=f"osb_{s_q}_{sb}")
                for q in range(4):
                    def go_w(sb=sb, q=q, s_q=s_q, ot=ot, xt2=xt2):
                        slot = next_slot()
                        for ec in range(2):
                            nc.tensor.matmul(
                                slot,
                                xt2[:, ec * 512 + sb * 128: ec * 512 + sb * 128 + 128],
                                wo_t[ec][:, q * 256:(q + 1) * 256],
                                start=(ec == 0), stop=(ec == 1),
                                skip_group_check=True,
                            )
                        evict_engine().tensor_copy(
                            ot[:, q * 256:(q + 1) * 256], slot)
                        if q == 3:
                            s_blk = s_q * 4 + sb
                            nc.sync.dma_start(
                                out_d[s_blk * 128:(s_blk + 1) * 128, :], ot[:])
                    push_extra(10**9, go_w)

        def wo_tail(s_q, x_sbs, xt2):
            # last s_q: QK PSUM banks are free — use them wide, pipeline
            # per s-block, and split evicts across DVE+ACT for latency.
            for sb in range(4):
                slot = next_slot()
                for ec in range(2):
                    nc.tensor.transpose(
                        slot[:, ec * 128:(ec + 1) * 128].bitcast(F32R),
                        x_sbs[sb][:, ec * 128:(ec + 1) * 128],
                        ident[:],
                    )
                dst = xt2[:].rearrange("p (ec s) -> p ec s", ec=2)[
                    :, :, sb * 128:(sb + 1) * 128]
                src = slot.rearrange("p (ec c) -> p ec c", ec=2)
                if sb % 2:
                    nc.scalar.copy(dst, src)
                else:
                    nc.vector.tensor_copy(dst, src)
                ot = outsb_p.tile([128, D], BF16, tag="outsb",
                                  name=f"osb_{s_q}_{sb}")
                for m in range(2):
                    scr = ps_qk.tile([128, 1536], F32, tag="ps_qk", bufs=2)
                    for ec in range(2):
                        nc.tensor.matmul(
                            scr[:, 0:512],
                            xt2[:, ec * 512 + sb * 128: ec * 512 + sb * 128 + 128],
                            wo_t[ec][:, m * 512:(m + 1) * 512],
                            start=(ec == 0), stop=(ec == 1),
                        )
                    nc.vector.tensor_copy(
                        ot[:, m * 512:m * 512 + 256], scr[:, 0:256])
                    nc.scalar.copy(
                        ot[:, m * 512 + 256:m * 512 + 512], scr[:, 256:512])
                    s_blk = s_q * 4 + sb
                    nc.sync.dma_start(
                        out_d[s_blk * 128:(s_blk + 1) * 128,
                              m * 512:(m + 1) * 512],
                        ot[:, m * 512:(m + 1) * 512])

        # ---------------- attention rounds -----------------------------------
        pv = ps_pv.tile([128, 512], F32, tag="ps_pv")

        # remaining projections ride along with the early rounds, ordered by
        # deadline: K1 (QK tile 3), V0 (PV tile 3), K2, V1, K3, V2, V3, then
        # the quad-1 halves of K0/Q0 (first needed by round 2 = pair 2).
        k_chunk_extras(1)
        k_chunk_extras(2)
        k_chunk_extras(3)
        proj_qk_extras(wq_t, qq, qch0, 0, quads_sel=(1,),
                       dl_fn=lambda half, quad: 22)
        v_chunk_extras(0)
        v_chunk_extras(1)
        v_chunk_extras(2)
        v_chunk_extras(3)

        # global PV queue: PV/normalize/WO trail the QK/exp stream by a
        # dynamic number of tiles so they never block it (the first rounds
        # need a deep lag while the input DMAs stream in).
        pv_queue = []          # (at, k, pair, r)
        x_sbs_by_sq = {}
        tail_state = {}

        def do_normalize(r_):
            s_q_, pair_ = r_ // 4, r_ % 4
            rcp = small_p.tile([128, 8], F32, tag="rcp", name=f"rcp_{r_}")
            pv3 = pv[:, 0:264].rearrange("p (a c) -> p a c", c=33)
            nc.vector.reciprocal(rcp[:], pv3[:, 0:8, 32])
            if pair_ == 0:
                x_sbs_by_sq[s_q_] = {
                    sb: xsb_p.tile([128, E], F32R, tag="xsb",
                                   name=f"xsb_{s_q_}_{sb}")
                    for sb in range(4)}
            x_sbs = x_sbs_by_sq[s_q_]
            for sb in range(4):
                for hl in range(2):
                    h = pair_ * 2 + hl
                    acc = hl * 4 + sb
                    if r_ == N_ROUNDS - 1 and hl == 1:
                        # ScalarE is idle after the last exp — use it
                        nc.scalar.mul(
                            x_sbs[sb][:, h * 32:(h + 1) * 32],
                            pv[:, acc * 33: acc * 33 + 32],
                            rcp[:, acc:acc + 1])
                    else:
                        nc.vector.tensor_scalar_mul(
                            x_sbs[sb][:, h * 32:(h + 1) * 32],
                            pv[:, acc * 33: acc * 33 + 32],
                            rcp[:, acc:acc + 1])
            if pair_ == 3:
                xt2 = xt_p.tile([128, 1024], BF16, tag="xt",
                                name=f"xt_{s_q_}")
                if r_ == N_ROUNDS - 1:
                    tail_state["xt2"] = xt2
                    tail_state["s_q"] = s_q_
                else:
                    wo_extras(s_q_, x_sbs, xt2)

        def pop_pv():
            at_, k_, pair_, r_ = pv_queue.pop(0)
            if k_ == 0:
                # zero the accumulator region; the PV matmuls accumulate
                # onto it (no start=True — see _pv_units)
                nc.vector.memset(pv[:, 0:264], 0.0)
            _pv_units(nc, pv, at_, k_, pair_, vaug)
            if k_ == TILES_PER_ROUND - 1:
                do_normalize(r_)

        gt = 0
        for r in range(N_ROUNDS):
            s_q, pair = r // 4, r % 4
            quad = pair // 2
            if pair == 0 and s_q + 1 < SC:
                q_chunk_extras(s_q + 1)

            for k in range(TILES_PER_ROUND):
                units = list(range(3 * k, min(3 * k + 3, UPR)))
                width = 512 * len(units)
                qkt = ps_qk.tile([128, 1536], F32, tag="ps_qk", bufs=2)
                for i, u in enumerate(units):
                    t_blk, hl = u // 2, u % 2
                    h = pair * 2 + hl
                    row = 32 * (h % 4)
                    nc.tensor.matmul(
                        qkt[:, i * 512:(i + 1) * 512],
                        kq[quad][row:row + 32, t_blk * 128:(t_blk + 1) * 128],
                        qq[quad][row:row + 32, s_q * 512:(s_q + 1) * 512],
                        start=True, stop=True,
                        tile_position=(row, 0),
                    )
                at = at_p.tile([128, 1536], BF16, tag="at", name=f"at_{r}_{k}")
                nc.scalar.activation(at[:, 0:width], qkt[:, 0:width], EXP,
                                     scale=SCALE)
                emit_extras_at(gt)
                pv_queue.append((at, k, pair, r))
                gt += 1
                target = WARM_LAG if gt < 22 else PV_LAG
                while len(pv_queue) > target:
                    pop_pv()

        while pv_queue:
            pop_pv()
        emit_extras(len(extras))
        wo_tail(tail_state["s_q"], x_sbs_by_sq[tail_state["s_q"]],
                tail_state["xt2"])


def _pv_units(nc, pv, at, k, pair, vaug):
    # NOTE: start=True clears the has_written state of the WHOLE PSUM bank,
    # so 8 independent accumulation groups in one bank must never use it.
    # The bank is zeroed by a DVE memset between rounds instead; accumulation
    # onto the zeros relies on the has_written bits staying set.
    units = list(range(3 * k, min(3 * k + 3, UPR)))
    for i, u in enumerate(units):
        t_blk, hl = u // 2, u % 2
        h = pair * 2 + hl
        for sb in range(4):
            acc = hl * 4 + sb
            nc.tensor.matmul(
                pv[:, acc * 33: acc * 33 + 33],
                at[:, i * 512 + sb * 128: i * 512 + sb * 128 + 128],
                vaug[t_blk][:, h * VSTRIDE: h * VSTRIDE + 33],
                start=False, stop=(t_blk == TC - 1),
                skip_group_check=True,
            )


def _get_nc():
    global _CACHED_NC
    if _CACHED_NC is None:
        _CACHED_NC = _build_nc()
    return _CACHED_NC


def kernel(q, k, v, Wq, Wk, Wv, Wo):
    import ml_dtypes
    bf16 = ml_dtypes.bfloat16

    q = np.asarray(q, dtype=np.float32)
    k = np.asarray(k, dtype=np.float32)
    v = np.asarray(v, dtype=np.float32)
    Wq = np.asarray(Wq, dtype=np.float32)
    Wk = np.asarray(Wk, dtype=np.float32)
    Wv = np.asarray(Wv, dtype=np.float32)
    Wo = np.asarray(Wo, dtype=np.float32)

    ident = np.eye(128, dtype=np.float32)

    in_maps = []
    for c in range(N_CORES):
        b = c // 2
        g = c % 2
        hs = slice(g * HPC, (g + 1) * HPC)
        wq_s = np.ascontiguousarray(
            Wq[hs].transpose(1, 0, 2).reshape(D, E)).astype(bf16)
        wk_s = np.ascontiguousarray(
            Wk[hs].transpose(1, 0, 2).reshape(D, E)).astype(bf16)
        wv_s = np.ascontiguousarray(
            Wv[hs].transpose(1, 0, 2).reshape(D, E)).astype(bf16)
        wo_s = np.ascontiguousarray(Wo[g * E:(g + 1) * E, :]).astype(bf16)
        in_maps.append({
            "qt": np.ascontiguousarray(q[b].T).astype(bf16),
            "kt": np.ascontiguousarray(k[b].T).astype(bf16),
            "vt": np.ascontiguousarray(v[b].T).astype(bf16),
            "wq": wq_s, "wk": wk_s, "wv": wv_s, "wo": wo_s,
            "ident": ident, "ones8": np.ones((128, HPC), dtype=bf16),
        })

    nc = _get_nc()
    res = None
    last_err = None
    for attempt in range(4):
        try:
            res = run_bass_kernel_spmd(nc, in_maps, core_ids=list(range(N_CORES)))
            break
        except Exception as e:  # transient axon worker recovery
            last_err = e
            import time as _time
            _time.sleep(15 * (attempt + 1))
    if res is None:
        raise last_err

    out = np.empty((B, S, D), dtype=np.float32)
    for b in range(B):
        out[b] = (np.asarray(res.results[2 * b]["out"], dtype=np.float32)
                  + np.asarray(res.results[2 * b + 1]["out"], dtype=np.float32))
    return out

